# revision 1
# baseline (speedup 1.0000x reference)
"""Trainium2 Bass kernel for the DiscMaker mkaarma/controller scan.

Math per step t (per batch element b):
    ns    = tanh(x_t @ Wx[j] + kstate @ Wh[j])          j=0..2   [B,3,S]
    enc   = tanh(x_t @ We + kstate @ Ue)                         [B,E]
    cst   = tanh([enc, err] @ Wi + cst @ Whc)                    [B,H]
    out   = cst @ Wo                                             [B,4]
    gate  = softmax(out[:, :3] @ Wd + bd) ; theta = sigmoid(out[:, 3])
    gate  = gate*theta + gate_prev*(1-theta)
    kstate= sum_j gate[:,j] * ns[:,j,:] ; pred = kstate[:,-1] ; err = pred - y_t

Device design (per core, batch shard b=32, feature-on-partition):
  - kstate is never materialized: we carry G[s,(j,b)] = gate[j,b]*ns[s,j,b]; all
    kstate consumers contract G over s with a single matmul whose PSUM out AP
    repeats (stride-0 j) so the 3 j-slices accumulate via has_written.
  - err is computed once per step on the otherwise-idle DVE
    (err = sum_j G[0,j,:] - y, with the S dim permuted so the pred feature
    sits on partition 0), feeding one rank-1 matmul per controller half.
  - theta via sigmoid(z) = (1+tanh(z/2))/2 so one ACT table set {tanh, exp}
    serves the whole loop. Gate head folded: Wfold = [Wo[:, :3]@Wd, 0.5*Wo[:,3]].
  - gate algebra on DVE in batch-on-partition [32,*] without materializing the
    softmax: d = e*r0 - g_prev ; s = e*r0 + g_prev ; g_new = th2*d + s, each a
    single scalar_tensor_tensor.
  - gate transpose+broadcast in one matmul per j: lhsT = g_new[:, j] broadcast
    along free (stride 0) against an I32 rhs -> gateB[s, b] = g_new[b, j].
  - dummy matmuls into a scratch PSUM bank keep the PE HAM busy through the
    gate phase so real matmuls run at 2.4 GHz instead of the cold 1.2 GHz.
  - preds come from G_all[0,:] which is DMA'd out once; host sums over j.
  - weights with K<128 are zero-padded to K=128 so FWL (fast weight load)
    engages: LDWEIGHTS 27ns instead of 107ns.
"""

import os
import sys

import numpy as np

sys.path.insert(0, "/opt/trn_rl_repo")

import concourse.bass as bass  # noqa: E402
import concourse.tile as tile  # noqa: E402
from concourse import bacc, mybir  # noqa: E402

F16 = mybir.dt.float16
F32 = mybir.dt.float32
AF = mybir.ActivationFunctionType
ALU = mybir.AluOpType

B, T, D_IN, S, E, H, NOUT = 256, 512, 64, 128, 128, 256, 4
NCORES = 8
BC = B // NCORES  # 32 batch per core

JUNK_MID = 0
JUNK_GATE = 0
JUNK_N = 256
ERR_DVE = True


def build_program(T_steps=T):
    nc = bacc.Bacc(
        "TRN2", target_bir_lowering=False, debug=False, enable_asserts=False
    )
    Tn = T_steps

    def din(name, shape, dt=F16):
        return nc.dram_tensor(name, shape, dt, kind="ExternalInput").ap()

    xw = din("xw", [S, Tn * BC])           # x[b,t,d] -> [d, t*32+b], d padded->128
    negy = din("negy", [1, Tn * BC])       # -y[b,t]  -> [1, t*32+b]
    gf20 = din("gf20", [S, 32])            # gate0 in rows 0:32 cols 0:3, else 0
    whcat = din("whcat", [S, 3 * S])       # Wh[j][s_in, s_out] stacked on free
    wxcat = din("wxcat", [S, 3 * S])       # Wx[j] zero-padded rows 64:128
    ue = din("ue", [S, E])
    we = din("we", [S, E])                 # We zero-padded rows 64:128
    wit0 = din("wit0", [E, S])             # Wi[0:128] cols 0:128
    wit1 = din("wit1", [E, S])             # Wi[0:128] cols 128:256
    wib = din("wib", [1, H])               # Wi[128]
    whc00 = din("whc00", [S, S])           # Whc[0:128, 0:128]
    whc01 = din("whc01", [S, S])           # Whc[0:128, 128:256]
    whc10 = din("whc10", [S, S])           # Whc[128:256, 0:128]
    whc11 = din("whc11", [S, S])           # Whc[128:256, 128:256]
    wfold = din("wfold", [S, 8])           # [F[0:128] | F[128:256]], F=[Wo@Wd_ext]
    bdext = din("bdext", [1, 4])           # [bd, 0]
    eye32 = din("eye32", [S, BC])          # 0.5*I_32 in rows 0:32, zeros below
    ones132 = din("ones132", [1, S])       # 1.0 in cols 0:32, 0 elsewhere

    g127o = nc.dram_tensor("g127", [1, Tn * 96], F16, kind="ExternalOutput").ap()

    with tile.TileContext(nc) as tc:
        import contextlib
        stk = contextlib.ExitStack()
        persist = stk.enter_context(tc.tile_pool(name="persist", bufs=1))

        def ptile(shape, dtype, name):
            return persist.tile(shape, dtype, name=name, tag=name)

        # ---- persistent SBUF ----
        s_xw = ptile([S, Tn * BC], F16, "s_xw")
        s_negy = ptile([1, Tn * BC], F16, "s_negy")
        s_whcat = ptile([S, 3 * S], F16, "s_whcat")
        s_wxcat = ptile([S, 3 * S], F16, "s_wxcat")
        s_ue = ptile([S, E], F16, "s_ue")
        s_we = ptile([S, E], F16, "s_we")
        s_wit = [ptile([E, S], F16, "s_wit0"), ptile([E, S], F16, "s_wit1")]
        s_wib = ptile([1, H], F16, "s_wib")
        s_whc = [[ptile([S, S], F16, "s_whc00"), ptile([S, S], F16, "s_whc01")],
                 [ptile([S, S], F16, "s_whc10"), ptile([S, S], F16, "s_whc11")]]
        s_wfold = ptile([S, 8], F16, "s_wfold")
        s_bdext = ptile([1, 4], F16, "s_bdext")
        s_eye = ptile([S, BC], F16, "s_eye")
        s_ones132 = ptile([1, S], F16, "s_ones132")
        s_gall = ptile([S, Tn * 96], F16, "s_gall")
        s_cst = ptile([S, 5 * BC], F16, "s_cst")
        gf2a = ptile([S, 32], F16, "gf2a")
        gf2b = ptile([S, 32], F16, "gf2b")

        for dst, src in [
            (s_xw, xw), (s_negy, negy), (s_whcat, whcat), (s_wxcat, wxcat),
            (s_ue, ue), (s_we, we), (s_wit[0], wit0), (s_wit[1], wit1),
            (s_wib, wib), (s_whc[0][0], whc00), (s_whc[0][1], whc01),
            (s_whc[1][0], whc10), (s_whc[1][1], whc11), (s_wfold, wfold),
            (s_bdext, bdext), (s_eye, eye32), (s_ones132, ones132),
            (gf2a, gf20),
        ]:
            nc.sync.dma_start(out=dst[:], in_=src)
        nc.vector.memset(gf2b[:], 0.0)
        nc.vector.memset(s_cst[:, 2 * BC:5 * BC], 0.0)

        # ---- pools ----
        pEnc = stk.enter_context(tc.tile_pool(name="pEnc", bufs=1, space="PSUM"))
        pNs = stk.enter_context(tc.tile_pool(name="pNs", bufs=2, space="PSUM"))
        pB = stk.enter_context(tc.tile_pool(name="pB", bufs=1, space="PSUM"))
        pG = stk.enter_context(tc.tile_pool(name="pG", bufs=1, space="PSUM"))
        pGB = stk.enter_context(tc.tile_pool(name="pGB", bufs=1, space="PSUM"))
        pJ = stk.enter_context(tc.tile_pool(name="pJ", bufs=1, space="PSUM"))
        wk = stk.enter_context(tc.tile_pool(name="wk", bufs=3))

        ts = bass.ts
        gf2 = [gf2a, gf2b]

        def junk(n):
            for _ in range(n):
                jt = pJ.tile([S, JUNK_N], F32, tag="junk")
                nc.tensor.matmul(jt[:], s_whcat[:, 0:S], s_whcat[:, 0:JUNK_N],
                                 start=True, stop=True)

        for t in range(Tn):
            xt = s_xw[:, ts(t, BC)]
            gprev = None if t == 0 else s_gall[:, ts(t - 1, 96)]

            # --- hgp = 0.5*gf2_prev, off-chain (inputs from step t-1)
            gp = gf2[t % 2]
            gn = gf2[(t + 1) % 2]
            hgp = wk.tile([BC, 3], F32, tag="hgp")
            nc.vector.tensor_scalar_mul(hgp[:], gp[0:BC, 0:3], 0.5)

            # --- err on DVE, available early (inputs are from step t-1)
            if t > 0 and ERR_DVE:
                red = wk.tile([1, BC], F32, tag="red")
                src = s_gall[0:1, (t - 1) * 96:t * 96]
                src3 = src.rearrange("p (j b) -> p b j", j=3)
                nc.vector.tensor_reduce(red[:], src3, mybir.AxisListType.X,
                                        ALU.add)
                errt = wk.tile([1, BC], F16, tag="errt")
                nc.vector.tensor_tensor(errt[:], red[:],
                                        s_negy[:, ts(t - 1, BC)], ALU.add)

            # --- stage A: enc + 3 candidate branches, pre-activation in PSUM
            a_enc = pEnc.tile([S, BC], F32, tag="a_enc")
            nc.tensor.matmul(a_enc[:], s_we[:], xt, start=True, stop=(t == 0))
            if t > 0:
                for j in range(3):
                    nc.tensor.matmul(a_enc[:], s_ue[:], gprev[:, ts(j, BC)],
                                     start=False, stop=(j == 2))
            a_ns = pNs.tile([S, 96], F32, tag="a_ns")
            for k in range(3):
                sl = a_ns[:, ts(k, BC)]
                nc.tensor.matmul(sl, s_wxcat[:, ts(k, S)], xt,
                                 start=True, stop=(t == 0))
                if t > 0:
                    for j in range(3):
                        nc.tensor.matmul(sl, s_whcat[:, ts(k, S)],
                                         gprev[:, ts(j, BC)],
                                         start=False, stop=(j == 2))

            # --- tanh(enc) first (on the critical chain)
            A_enc = wk.tile([S, BC], F16, tag="A_enc")
            nc.scalar.activation(A_enc[:], a_enc[:], AF.Tanh)

            # --- controller pre-activation [128, 64] (two halves on free)
            b_ = pB.tile([S, 2 * BC], F32, tag="b_")
            for h in range(2):
                sl = b_[:, ts(h, BC)]
                first = True
                if t > 0:
                    nc.tensor.matmul(sl, s_whc[0][h][:], s_cst[:, 0:BC],
                                     start=True, stop=False)
                    nc.tensor.matmul(sl, s_whc[1][h][:], s_cst[:, BC:2 * BC],
                                     start=False, stop=False)
                    if ERR_DVE:
                        nc.tensor.matmul(sl, s_wib[:, ts(h, S)], errt[:],
                                         start=False, stop=False)
                    else:
                        wib_l = s_wib[:, ts(h, S)]
                        for j in range(3):
                            nc.tensor.matmul(
                                sl, wib_l,
                                s_gall[0:1, (t - 1) * 96 + j * BC:(t - 1) * 96 + (j + 1) * BC],
                                start=False, stop=False)
                        nc.tensor.matmul(sl, wib_l, s_negy[:, ts(t - 1, BC)],
                                         start=False, stop=False)
                    first = False
                nc.tensor.matmul(sl, s_wit[h][:], A_enc[:],
                                 start=first, stop=True)
            junk(JUNK_MID)

            # --- tanh -> cstate (fp16, feeds matmuls)
            nc.scalar.activation(s_cst[:, 0:2 * BC], b_[:], AF.Tanh)

            # --- gate head: gate_pre [32, 4] = cst @ Wfold + bd_ext
            #     lhsT padded to 128 cols (FWL); rows 32:128 of g are garbage
            g = pG.tile([S, 4], F32, tag="g")
            nc.tensor.matmul(g[:], s_ones132[:], s_bdext[:],
                             start=True, stop=False)
            nc.tensor.matmul(g[:], s_cst[:, 0:4 * BC], s_wfold[:, 0:4],
                             start=False, stop=False)
            nc.tensor.matmul(g[:], s_cst[:, BC:5 * BC], s_wfold[:, 4:8],
                             start=False, stop=True)
            junk(JUNK_GATE)

            # --- exp + row-sum, th2 = tanh(o3/2)
            e = wk.tile([BC, 3], F32, tag="e")
            z = wk.tile([BC, 1], F32, tag="z")
            nc.scalar.activation(e[:], g[0:BC, 0:3], AF.Exp, accum_out=z[:])
            th2 = wk.tile([BC, 1], F32, tag="th2")
            nc.scalar.activation(th2[:], g[0:BC, 3:4], AF.Tanh)
            # ns tanh emitted here: off the critical path until the G multiply
            A_ns = wk.tile([S, 96], F16, tag="A_ns")
            nc.scalar.activation(A_ns[:], a_ns[:], AF.Tanh)

            # --- gate algebra on DVE (carry gf2 = 2*gate):
            #     d = e*r0 - hgp ; s = e*r0 + hgp ; gf2' = th2*d + s
            r0 = wk.tile([BC, 1], F32, tag="r0")
            nc.vector.reciprocal(r0[:], z[:])
            dd = wk.tile([BC, 3], F32, tag="dd")
            nc.vector.scalar_tensor_tensor(dd[:], e[:], r0[:], hgp[:],
                                           ALU.mult, ALU.subtract)
            ss = wk.tile([BC, 3], F32, tag="ss")
            nc.vector.scalar_tensor_tensor(ss[:], e[:], r0[:], hgp[:],
                                           ALU.mult, ALU.add)
            nc.vector.scalar_tensor_tensor(gn[0:BC, 0:3], dd[:], th2[:], ss[:],
                                           ALU.mult, ALU.add)

            # --- transpose+broadcast in one matmul per j:
            #     gb[s, b] = sum_k gn[k, j] * I32[k, b] = gn[b, j]
            gb = pGB.tile([S, 96], F32, tag="gb")
            for j in range(3):
                nc.tensor.matmul(gb[:, ts(j, BC)],
                                 gn[:, j:j + 1].broadcast_to([S, S]),
                                 s_eye[:], start=True, stop=True)
            nc.vector.tensor_mul(s_gall[:, ts(t, 96)], A_ns[:], gb[:])

        nc.sync.dma_start(out=g127o, in_=s_gall[0:1, :])
        stk.close()
    nc.finalize()
    return nc


# ---------------- host side ----------------

def _pack_inputs(x, y, Wx, Wh, We, Ue, Wi, Whc, Wo, Wd, bd, gate0, Tn=T):
    """Build the 8 per-core input dicts."""
    f16 = np.float16
    F = np.concatenate(
        [Wo[:, :3] @ Wd, 0.5 * Wo[:, 3:4]], axis=1
    ).astype(np.float32)  # [256, 4]
    # permute the S dim so the prediction feature (s=127) sits on partition 0
    # (matmul operands must have base partition 0/32/64)
    perm = np.arange(S)
    perm[[0, S - 1]] = [S - 1, 0]
    Whp = [Wh[j][perm][:, perm] for j in range(3)]
    Wxp = [Wx[j][:, perm] for j in range(3)]

    def padk(a):  # zero-pad contraction dim to 128 rows (FWL eligibility)
        out = np.zeros((S, a.shape[1]), np.float32)
        out[:a.shape[0]] = a
        return out

    eye = np.zeros((S, BC), np.float32)
    eye[0:BC, 0:BC] = 0.5 * np.eye(BC)
    shared = {
        "whcat": np.concatenate(Whp, axis=1).astype(f16),
        "wxcat": padk(np.concatenate(Wxp, axis=1)).astype(f16),
        "ue": Ue[perm, :].astype(f16),
        "we": padk(We).astype(f16),
        "wit0": Wi[0:E, 0:S].astype(f16),
        "wit1": Wi[0:E, S:2 * S].astype(f16),
        "wib": Wi[E:E + 1].astype(f16),
        "whc00": Whc[0:S, 0:S].astype(f16),
        "whc01": Whc[0:S, S:2 * S].astype(f16),
        "whc10": Whc[S:2 * S, 0:S].astype(f16),
        "whc11": Whc[S:2 * S, S:2 * S].astype(f16),
        "wfold": np.concatenate([F[0:S], F[S:2 * S]], axis=1).astype(f16),
        "bdext": np.concatenate([bd, [0.0]]).reshape(1, 4).astype(f16),
        "eye32": eye.astype(f16),
        "ones132": np.concatenate(
            [np.ones((1, BC)), np.zeros((1, S - BC))], axis=1).astype(f16),
    }
    in_maps = []
    for c in range(NCORES):
        bs = slice(c * BC, (c + 1) * BC)
        xs = x[bs, :Tn]                      # [32, T, 64]
        ys = y[bs, :Tn]                      # [32, T]
        g0 = gate0[bs]                       # [32, 3]
        gf20 = np.zeros((S, 32), np.float32)
        gf20[0:BC, 0:3] = 2.0 * g0
        xwp = np.zeros((S, Tn * BC), np.float32)
        xwp[0:D_IN] = xs.transpose(2, 1, 0).reshape(D_IN, Tn * BC)
        m = dict(shared)
        m["xw"] = xwp.astype(f16)
        m["negy"] = np.ascontiguousarray(
            (-ys.T).reshape(1, Tn * BC)
        ).astype(f16)
        m["gf20"] = gf20.astype(f16)
        in_maps.append(m)
    return in_maps


_PROG_CACHE = {}
LAST_RESULT = {}


def kernel(x, y, Wx, Wh, We, Ue, Wi, Whc, Wo, Wd, bd, gate0):
    from concourse.bass_utils import run_bass_kernel_spmd

    args = [np.asarray(a, dtype=np.float32) for a in
            (x, y, Wx, Wh, We, Ue, Wi, Whc, Wo, Wd, bd, gate0)]
    in_maps = _pack_inputs(*args)
    if "prog" not in _PROG_CACHE:
        _PROG_CACHE["prog"] = build_program(T)
    nc = _PROG_CACHE["prog"]
    trace = bool(int(os.environ.get("TRN_KERNEL_TRACE", "0")))
    res = run_bass_kernel_spmd(
        nc, in_maps, core_ids=list(range(NCORES)), trace=trace
    )
    LAST_RESULT["exec_time_ns"] = res.exec_time_ns
    LAST_RESULT["res"] = res
    preds = np.zeros((B, T), np.float32)
    for c in range(NCORES):
        g127 = res.results[c]["g127"].reshape(T, 3, BC).astype(np.float32)
        preds[c * BC:(c + 1) * BC] = g127.sum(axis=1).T
    return preds

